# revision 28
# baseline (speedup 1.0000x reference)
"""DVAE encoder (batched DAG GRU message passing) on 8 trn2 NeuronCores.

v3 strategy: pure data-parallel over batch (256 graphs/core). Feature-major
compute (features on partitions, batch on free dim); GRU/gate/mapper are
weight-stationary matmuls with N=256. The r,z gate GEMM runs in fp8-e4m3
DoubleRow (sigmoid paths absorb the quantization; verified well under the
accuracy budget).

Predecessor aggregation runs on the PE as diag-mask matmuls accumulating in
PSUM, with all diagonal masks prebuilt on the host and DMAed in (no
on-device mask builds). The per-vertex mask chain is split into two batches
scheduled into the PE's two natural idle windows: the mid-step GRU-pointwise
bubble and the step-boundary wait for the next hidden state, keeping the PE
continuously busy (HAM stays unthrottled). Sigmoids/tanh and PSUM->SBUF
copies run on the Scalar engine; the n/h'/message pointwise runs on DVE.
"""

import numpy as np

B, MAX_N, NVT, HS, NZ = 2048, 16, 8, 501, 56
HP = 512
NC_CORES = 8
BL = B // NC_CORES   # 256 per core
NBT = BL // 128      # 2 batch tiles

FP8_WA = False       # r,z GEMM in fp8 DoubleRow (off: bf16, same PE cost)

_CACHE = {}

# host-side mask index: (w, u, bt) -> column block, w-major so the DMA
# arrives in first-use order
def _mask_index():
    idx = {}
    k = 0
    for w in range(1, MAX_N):
        for u in range(w):
            for bt in range(NBT):
                idx[(w, u, bt)] = k
                k += 1
    return idx, k

MASK_IDX, N_MASKS = _mask_index()


def _build_nc():
    import concourse.mybir as mybir
    import concourse.tile as tile
    from concourse import bacc

    F32 = mybir.dt.float32
    BF = mybir.dt.bfloat16
    F8 = mybir.dt.float8e4
    DR = mybir.MatmulPerfMode.DoubleRow
    Sig = mybir.ActivationFunctionType.Sigmoid
    Tanh = mybir.ActivationFunctionType.Tanh
    Ident = mybir.ActivationFunctionType.Identity
    ADD = mybir.AluOpType.add
    MUL = mybir.AluOpType.mult

    nc = bacc.Bacc("TRN2", target_bir_lowering=False, debug=False,
                   num_devices=NC_CORES)

    # ---- DRAM parameters (host-prepped) ----
    d_wa = nc.dram_tensor("wa", [128, 4 * 1024], F8 if FP8_WA else BF,
                          kind="ExternalInput").ap()
    d_wb = nc.dram_tensor("wb", [128, 512], BF, kind="ExternalInput").ap()
    d_wc = nc.dram_tensor("wc", [128, 4 * 512], BF, kind="ExternalInput").ap()
    d_wg = nc.dram_tensor("wg", [128, 4 * 512], BF, kind="ExternalInput").ap()
    d_wm = nc.dram_tensor("wm", [128, 4 * 512], BF, kind="ExternalInput").ap()
    d_wf = nc.dram_tensor("wf", [128, 4 * 112], BF, kind="ExternalInput").ap()
    d_gb = nc.dram_tensor("gb", [128, 64], F32, kind="ExternalInput").ap()
    d_mb = nc.dram_tensor("mb", [128, 64], F32, kind="ExternalInput").ap()
    d_fcb = nc.dram_tensor("fcb", [128, 1], F32, kind="ExternalInput").ap()
    d_xh = nc.dram_tensor("xh", [128, MAX_N * NBT * 9], BF,
                          kind="ExternalInput").ap()
    d_dm = nc.dram_tensor("dmasks", [128, N_MASKS * 128], BF,
                          kind="ExternalInput").ap()
    d_id = nc.dram_tensor("ident", [128, 128], BF, kind="ExternalInput").ap()
    d_y = nc.dram_tensor("y", [112, BL], F32, kind="ExternalOutput").ap()

    with tile.TileContext(nc) as tc:
        with tc.tile_pool(name="wts", bufs=1) as wts, \
             tc.tile_pool(name="state", bufs=1) as state, \
             tc.tile_pool(name="hbuf", bufs=2) as hbuf, \
             tc.tile_pool(name="work", bufs=2) as work, \
             tc.tile_pool(name="gps", bufs=2, space="PSUM") as gps, \
             tc.tile_pool(name="pgg", bufs=1, space="PSUM") as paggp, \
             tc.tile_pool(name="ptr", bufs=2, space="PSUM") as ptrp:

            # ---- load static data ----
            wa = wts.tile([128, 4, 1024], F8 if FP8_WA else BF, tag="wa",
                          name="wa")
            wb = wts.tile([128, 512], BF, tag="wb", name="wb")
            wc = wts.tile([128, 4, 512], BF, tag="wc", name="wc")
            wg = wts.tile([128, 4, 512], BF, tag="wg", name="wg")
            wm = wts.tile([128, 4, 512], BF, tag="wm", name="wm")
            wf = wts.tile([128, 4, 112], BF, tag="wf", name="wf")
            gb = wts.tile([128, 64], F32, tag="gb", name="gb")
            mb = wts.tile([128, 64], F32, tag="mb", name="mb")
            fcb = wts.tile([128, 1], F32, tag="fcb", name="fcb")
            xh = wts.tile([128, MAX_N * NBT * 9], BF, tag="xh", name="xh")
            dmasks = wts.tile([128, N_MASKS * 128], BF, tag="dmasks",
                              name="dmasks")
            ident = wts.tile([128, 128], BF, tag="ident", name="ident")
            # step-0-critical tensors first
            for t, d in ((xh, d_xh), (ident, d_id), (wa, d_wa), (wb, d_wb),
                         (wc, d_wc), (gb, d_gb), (mb, d_mb), (wg, d_wg),
                         (wm, d_wm), (wf, d_wf), (fcb, d_fcb)):
                nc.sync.dma_start(out=t[:], in_=d[:])
            # masks arrive in per-vertex chunks so early steps don't wait
            # on the whole 7.9MB transfer
            for w in range(1, MAX_N):
                k0 = MASK_IDX[(w, 0, 0)]
                k1 = MASK_IDX[(w, w - 1, NBT - 1)] + 1
                nc.sync.dma_start(out=dmasks[:, k0 * 128:k1 * 128],
                                  in_=d_dm[:, k0 * 128:k1 * 128])

            # messages batch-major: [128, v, bt, 512]
            msb = state.tile([128, MAX_N, NBT, 512], BF, tag="msb",
                             name="msb")

            def mask_ap(w, u, bt):
                k = MASK_IDX[(w, u, bt)]
                return dmasks[:, k * 128:(k + 1) * 128]

            def make_hT(v, hin):
                """Transpose batch-major h_in (x rows already merged) to the
                feature-major hT used by the GRU matmuls."""
                # overwrite x one-hot + ones columns (baseline data path)
                for bt in range(NBT):
                    eng = nc.vector if bt == 0 else nc.scalar
                    if eng is nc.vector:
                        eng.tensor_copy(
                            hin[:, bt, 501:510],
                            xh[:, (v * NBT + bt) * 9:(v * NBT + bt) * 9 + 9])
                    else:
                        eng.copy(
                            hin[:, bt, 501:510],
                            xh[:, (v * NBT + bt) * 9:(v * NBT + bt) * 9 + 9])
                ptp = ptrp.tile([128, 2, 4, 128], BF, tag="ptr", name="ptp")
                for bt in range(NBT):
                    for kc in range(4):
                        nc.tensor.transpose(
                            ptp[:, bt, kc, :],
                            hin[:, bt, kc * 128:(kc + 1) * 128], ident[:])
                t = hbuf.tile([128, 4, 256], BF, tag="hT", name="hT")
                for bt in range(NBT):
                    # kc-split across DVE and Scalar so neither queue blocks
                    nc.vector.tensor_copy(t[:, 0:2, bt * 128:(bt + 1) * 128],
                                          ptp[:, bt, 0:2, :])
                    nc.scalar.copy(t[:, 2:4, bt * 128:(bt + 1) * 128],
                                   ptp[:, bt, 2:4, :])
                if FP8_WA:
                    t8 = hbuf.tile([128, 4, 256], F8, tag="hT8", name="hT8")
                    nc.vector.tensor_copy(t8[:, 0:2, :], t[:, 0:2, :])
                    nc.vector.tensor_copy(t8[:, 2:4, :], t[:, 2:4, :])
                else:
                    t8 = t
                return t, t8

            hin0 = work.tile([128, 2, 512], BF, tag="hin", name="hin")
            nc.vector.memset(hin0[:], 0.0)
            hT, hT8 = make_hT(0, hin0)

            # pagg: one live PSUM accumulator for vertex vn, filled entirely
            # during step vn-1 in three slices that plug the PE's wait
            # windows (mid-step, post-WG/WM, and the final msg term).
            def agg_emit(p, vn, us, is_final):
                if vn >= MAX_N or not us:
                    return
                for bt in range(NBT):
                    for u in us:
                        nc.tensor.matmul(
                            p[:, bt, :, :], mask_ap(vn, u, bt),
                            msb[:, u, bt, :],
                            start=(u == 0), stop=(is_final and u == us[-1]),
                            skip_group_check=True)

            for v in range(MAX_N):
                # ---- GRU GEMMs in two mt-pair waves ----
                rz = []
                pbcs = []
                for mtp in range(2):
                    pa = gps.tile([128, 2, 2, 256], F32, tag="gemm",
                                  name="pa")
                    if FP8_WA:
                        # i-major: the first wave only needs hT8 kc-pair 0,
                        # which is ready before the full hT cast completes
                        for i in range(2):
                            for mt2 in range(2):
                                mt = 2 * mtp + mt2
                                for half in range(2):
                                    co = half * 512 + mt * 128
                                    nc.tensor.matmul(
                                        pa[:, mt2, half, :],
                                        wa[:, 2 * i:2 * i + 2, co:co + 128],
                                        hT8[:, 2 * i:2 * i + 2, :],
                                        start=(i == 0), stop=(i == 1),
                                        perf_mode=DR)
                    else:
                        for kc in range(4):
                            for mt2 in range(2):
                                mt = 2 * mtp + mt2
                                for half in range(2):
                                    co = half * 512 + mt * 128
                                    nc.tensor.matmul(
                                        pa[:, mt2, half, :],
                                        wa[:, kc, co:co + 128],
                                        hT[:, kc, :],
                                        start=(kc == 0), stop=(kc == 3))
                    r_t = work.tile([128, 2, 2, 256], BF, tag=f"rz{mtp}",
                                    name="rz")
                    nc.scalar.activation(r_t[:], pa[:], Sig)
                    rz.append(r_t)

                    pbc = gps.tile([128, 2, 2, 256], F32, tag="gemm",
                                   name="pbc")
                    pbcs.append(pbc)
                    for mt2 in range(2):
                        mt = 2 * mtp + mt2
                        nc.tensor.matmul(
                            pbc[:, mt2, 0, :], wb[:, mt * 128:mt * 128 + 128],
                            hT[:, 3, :], start=True, stop=True)
                        for kc in range(4):
                            nc.tensor.matmul(
                                pbc[:, mt2, 1, :],
                                wc[:, kc, mt * 128:mt * 128 + 128],
                                hT[:, kc, :], start=(kc == 0), stop=(kc == 3))

                # mid-step bubble fill: first slice of the aggregation for
                # the next vertex (PE waits on hv here otherwise)
                vn = v + 1
                pagg = None
                s = 0
                if vn < MAX_N:
                    pagg = paggp.tile([128, 2, 2, 256], F32, tag="pagg",
                                      name="pagg")
                    s = max(0, (2 * (vn - 1)) // 3)
                    agg_emit(pagg, vn, list(range(0, s)), False)

                # ---- n-path pointwise + tanh ----
                n_t = work.tile([128, 4, 256], BF, tag="n_t", name="n_t")
                for mtp in range(2):
                    u_t = work.tile([128, 2, 256], BF, tag=f"u{mtp}",
                                    name="u")
                    t_t = work.tile([128, 2, 256], BF, tag=f"t{mtp}",
                                    name="t")
                    nc.vector.tensor_mul(u_t[:], rz[mtp][:, :, 0, :],
                                         pbcs[mtp][:, :, 1, :])
                    nc.vector.tensor_add(t_t[:], u_t[:],
                                         pbcs[mtp][:, :, 0, :])
                    nc.scalar.activation(n_t[:, 2 * mtp:2 * mtp + 2, :],
                                         t_t[:], Tanh)

                # ---- h' = n + z*(h-n) on DVE ----
                hv = work.tile([128, 4, 256], BF, tag="hv", name="hv")
                d_t = work.tile([128, 4, 256], BF, tag="d_t", name="d_t")
                e_t = work.tile([128, 4, 256], BF, tag="e_t", name="e_t")
                for mtp in range(2):
                    ks = slice(2 * mtp, 2 * mtp + 2)
                    nc.vector.tensor_sub(d_t[:, ks, :], hT[:, ks, :],
                                         n_t[:, ks, :])
                    nc.vector.tensor_mul(e_t[:, ks, :], rz[mtp][:, :, 1, :],
                                         d_t[:, ks, :])
                    nc.vector.tensor_add(hv[:, ks, :], e_t[:, ks, :],
                                         n_t[:, ks, :])

                # ---- gate / mapper GEMMs (kc-outer for early start) ----
                pgm = [gps.tile([128, 2, 2, 256], F32, tag="gemm",
                                name="pgm") for _ in range(2)]
                for kc in range(4):
                    for mt in range(4):
                        nc.tensor.matmul(
                            pgm[mt // 2][:, mt % 2, 0, :],
                            wg[:, kc, mt * 128:mt * 128 + 128],
                            hv[:, kc, :], start=(kc == 0), stop=(kc == 3))
                        nc.tensor.matmul(
                            pgm[mt // 2][:, mt % 2, 1, :],
                            wm[:, kc, mt * 128:mt * 128 + 128],
                            hv[:, kc, :], start=(kc == 0), stop=(kc == 3))

                g_t = work.tile([128, 4, 256], BF, tag="g_t", name="g_t")
                gm = work.tile([128, 4, 256], BF, tag="gm", name="gm")
                for mt in range(4):
                    nc.scalar.activation(
                        g_t[:, mt, :], pgm[mt // 2][:, mt % 2, 0, :], Sig,
                        bias=gb[:, mt * 16 + v:mt * 16 + v + 1])
                for mt in range(4):
                    nc.vector.scalar_tensor_tensor(
                        out=gm[:, mt, :], in0=pgm[mt // 2][:, mt % 2, 1, :],
                        scalar=mb[:, mt * 16 + v:mt * 16 + v + 1],
                        in1=g_t[:, mt, :], op0=ADD, op1=MUL)

                # post-WG/WM fill: remaining prefix terms run while the PE
                # waits for g/gm (they only need messages from steps < v)
                if vn < MAX_N:
                    agg_emit(pagg, vn, list(range(s, vn - 1)), False)

                # ---- transpose msg to batch-major ----
                ptg = ptrp.tile([128, 2, 4, 128], BF, tag="ptr", name="ptg")
                for bt in range(NBT):
                    for mt in range(4):
                        nc.tensor.transpose(
                            ptg[:, bt, mt, :],
                            gm[:, mt, bt * 128:(bt + 1) * 128], ident[:])
                for bt in range(NBT):
                    nc.vector.tensor_copy(msb[:, v, bt, :], ptg[:, bt, :, :])

                if vn < MAX_N:
                    # final aggregation term for vertex vn (uses msg v)
                    agg_emit(pagg, vn, [v], True)

                    # h_in(vn): PSUM -> SBUF, merge x, transpose
                    hin = work.tile([128, 2, 512], BF, tag="hin", name="hin")
                    for bt in range(NBT):
                        nc.scalar.copy(hin[:, bt, :], pagg[:, bt, :, :])
                    hT, hT8 = make_hT(vn, hin)

                # ---- final FC ----
                if v == MAX_N - 1:
                    pf = gps.tile([128, 2, 2, 256], F32, tag="gemm",
                                  name="pf")
                    for kc in range(4):
                        nc.tensor.matmul(
                            pf[:112, 0, 0, :], wf[:, kc, :112],
                            hv[:, kc, :], start=(kc == 0), stop=(kc == 3))
                    out_sb = work.tile([128, 256], F32, tag="out_sb",
                                       name="out_sb")
                    nc.scalar.activation(
                        out_sb[:112, :], pf[:112, 0, 0, :], Ident,
                        bias=fcb[:112, :])
                    nc.sync.dma_start(out=d_y[:], in_=out_sb[:112, :])

    nc.compile()
    return nc


def _prep_static(w_ih, w_hh, b_ih, b_hh, gate_w, gate_b, map_w,
                 fc1_w, fc1_b, fc2_w, fc2_b):
    import ml_dtypes
    f32 = np.float32
    bf16 = ml_dtypes.bfloat16
    fp8 = ml_dtypes.float8_e4m3
    bias = (b_ih + b_hh).astype(f32)
    WA = np.zeros((512, 1024), f32)
    WA[0:501, 0:501] = w_hh[0:501].T
    WA[501:509, 0:501] = w_ih[0:501].T
    WA[509, 0:501] = bias[0:501]
    WA[0:501, 512:1013] = w_hh[501:1002].T
    WA[501:509, 512:1013] = w_ih[501:1002].T
    WA[509, 512:1013] = bias[501:1002]
    WC = np.zeros((512, 512), f32)
    WC[0:501, 0:501] = w_hh[1002:1503].T
    WC[509, 0:501] = b_hh[1002:1503]
    WB = np.zeros((128, 512), f32)
    WB[117:125, 0:501] = w_ih[1002:1503].T
    WB[125, 0:501] = b_ih[1002:1503]
    WG = np.zeros((512, 512), f32)
    WG[0:501, 0:501] = gate_w[:, 0:501].T
    WM = np.zeros((512, 512), f32)
    WM[0:501, 0:501] = map_w[:, 0:501].T
    WF = np.zeros((512, 112), f32)
    WF[0:501, 0:56] = fc1_w.T
    WF[0:501, 56:112] = fc2_w.T

    def ktile_flat(W, cols, dt):
        return np.ascontiguousarray(
            W.reshape(4, 128, cols).transpose(1, 0, 2).reshape(128, 4 * cols)
        ).astype(dt)

    wa = ktile_flat(WA, 1024, fp8 if FP8_WA else bf16)
    wcf = ktile_flat(WC, 512, bf16)
    wgf = ktile_flat(WG, 512, bf16)
    wmf = ktile_flat(WM, 512, bf16)
    wff = ktile_flat(WF, 112, bf16)

    gbm = np.zeros((128, 64), f32)
    mbm = np.zeros((128, 64), f32)
    for mt in range(4):
        f0 = mt * 128
        n_real = max(0, min(128, 501 - f0))
        if n_real > 0:
            rows = np.arange(f0, f0 + n_real)
            gbm[:n_real, mt * 16:(mt + 1) * 16] = (
                gate_b[rows, None] + gate_w[rows, HS:HS + 16])
            mbm[:n_real, mt * 16:(mt + 1) * 16] = map_w[rows, HS:HS + 16]
    fcb = np.zeros((128, 1), f32)
    fcb[0:56, 0] = fc1_b
    fcb[56:112, 0] = fc2_b
    ident = np.eye(128, dtype=np.float32).astype(bf16)
    return dict(wa=wa, wb=WB.astype(bf16), wc=wcf, wg=wgf, wm=wmf,
                wf=wff, gb=gbm, mb=mbm, fcb=fcb, ident=ident)


def _prep_core(node_types, adj, core):
    import ml_dtypes
    f32 = np.float32
    bf16 = ml_dtypes.bfloat16
    off = core * BL
    nt = node_types[off:off + BL]          # [256, 16] int32
    ad = adj[off:off + BL].astype(f32)     # [256, 16, 16]
    # batch-major x blocks: 8 one-hot cols + a ones col per (v, bt)
    xh = np.zeros((128, MAX_N * NBT * 9), f32)
    for bt in range(NBT):
        nb = nt[bt * 128:(bt + 1) * 128]   # [128, 16]
        oh = (nb[:, :, None] == np.arange(NVT)[None, None, :]).astype(f32)
        for v in range(MAX_N):
            base = (v * NBT + bt) * 9
            xh[:, base:base + 8] = oh[:, v, :]
            xh[:, base + 8] = 1.0
    # prebuilt diagonal masks for the PE aggregation
    dm = np.zeros((128, N_MASKS * 128), f32)
    rng = np.arange(128)
    for (w, u, bt), k in MASK_IDX.items():
        dm[rng, k * 128 + rng] = ad[bt * 128:(bt + 1) * 128, w, u]
    return dict(xh=xh.astype(bf16), dmasks=dm.astype(bf16))


def kernel(node_types, adj, w_ih, w_hh, b_ih, b_hh, gate_w, gate_b, map_w,
           fc1_w, fc1_b, fc2_w, fc2_b):
    from concourse.bass_utils import run_bass_kernel_spmd

    if "nc" not in _CACHE:
        _CACHE["nc"] = _build_nc()
    nc = _CACHE["nc"]

    node_types = np.asarray(node_types)
    adj = np.asarray(adj, dtype=np.float32)
    static = _prep_static(
        np.asarray(w_ih, np.float32), np.asarray(w_hh, np.float32),
        np.asarray(b_ih, np.float32), np.asarray(b_hh, np.float32),
        np.asarray(gate_w, np.float32), np.asarray(gate_b, np.float32),
        np.asarray(map_w, np.float32),
        np.asarray(fc1_w, np.float32), np.asarray(fc1_b, np.float32),
        np.asarray(fc2_w, np.float32), np.asarray(fc2_b, np.float32))
    in_maps = []
    for c in range(NC_CORES):
        m = dict(static)
        m.update(_prep_core(node_types, adj, c))
        in_maps.append(m)

    res = run_bass_kernel_spmd(nc, in_maps, core_ids=list(range(NC_CORES)))
    ys = [res.results[c]["y"] for c in range(NC_CORES)]   # each [112, 256]
    out = np.concatenate(ys, axis=1).T                     # [2048, 112]
    return np.ascontiguousarray(out.astype(np.float32))
